# revision 7
# baseline (speedup 1.0000x reference)
"""Fused Linear + GroupNorm + Hardtanh kernel for Trainium2 (8 NeuronCores).

Problem: out = clip(groupnorm(x @ W.T + b, 32 groups), -2, 2), with
x [65536, 512] fp32, W [1024, 512] fp32, gamma=1/beta=0.

Strategy (data-parallel over the 8 cores, 8192 rows each):
 - Host pre-transposes x and casts matmul operands to fp16 (PSUM accum
   stays fp32); each core streams x.T tiles as the stationary operand,
   W.T stays SBUF-resident as the moving operand.
 - Group sums of y come from a second, *transposed* stats matmul
   (stationary = per-k-tile group-summed weights pre-scaled by -1/32,
   moving = 512 x columns, i.e. one matmul per 4 m-tiles), so the
   [-mean | 1] stationary needed by the mean/bias injection comes out
   of PSUM already in [group, m] layout.  The constant ones rows are
   added by the PSUM->SBUF staging copy (activation Identity with a
   per-partition bias mask).  Bias enters via the injection's ones row
   as b'' = b - groupmean(b).
 - The injection (rank-17 matmul per N-half) lands (b - mean) into the
   y PSUM, so the epilogue is: square per half (Scalar, fp16 out) ->
   first fp16 pair-fold on GpSimd (otherwise idle), second fold +
   segmented reduce (fp16) on Vector -> sqrt (Scalar) -> one fused
   one-Newton-reciprocal + scale + hardtanh-clip custom DVE op per
   half writing fp16 -> DMA (sync queue).
 - y PSUM is split into independent h0/h1 pools (3 x 1 bank each,
   + 2 stats banks = 8) with [h0 x4][stats][h1 x4] matmul emission.
   Per-engine emission runs oldest-pipeline-stage first.
 - Startup: a few zero warmup matmuls start the PE p-state ramp while
   the first DMAs land; weights and x chunks are packed so only a few
   large DMAs (fewer ~620ns queue dispatches) gate the first tiles;
   x streams as one packed [128, 4096] DMA per 1024-row chunk.
 - Output is written fp16 and widened to fp32 on the host.
"""
import sys

sys.path.insert(0, "/opt/trn_rl_repo")

import numpy as np

M_FULL, K, N = 65536, 512, 1024
NG, GS = 32, 32
EPS = 1e-5
HT = 2.0
N_CORES = 8
KT = K // 128  # 4 k-tiles
CHUNK = 1024  # x.T columns loaded per DMA chunk (8 m-tiles)
GRP = 4  # m-tiles per stats matmul group (512 moving cols)
SW = 49  # stats width: [16 groups | ones | pad...] @0, [16 groups | ones] @32
N_WARM = 4  # zero matmuls that start the PE p-state ramp during startup

_custom_ops = {}


def _register_custom_ops():
    """Add the fused scale+clip DVE op to the custom-op table (idempotent)."""
    if _custom_ops:
        return _custom_ops
    import concourse.dve_ops as dve_ops
    from concourse.dve_spec import Spec, Src0, Src1, C0, C1, C2, Zero, minn, \
        maxx, lower, _has_src1
    from concourse.dve_uop import DveOpSpec

    def register(name, spec):
        if name in dve_ops._SUB_OPCODE_FOR_NAME:
            return next(o for o in dve_ops.OPS if o.name == name)
        row = max(dve_ops._SUB_OPCODE_FOR_NAME.values()) + 1
        assert row < 0x20
        op = dve_ops.DveOp(name, spec, subdim=False, uops_sha={})
        dve_ops.OPS.append(op)
        dve_ops._SUB_OPCODE_FOR_NAME[name] = row
        dve_ops.CUSTOM_DVE_SPECS[name] = spec
        for ver in ("v3", "v4"):
            uops = lower(spec, ver=ver)
            op.uops_sha[ver] = DveOpSpec(
                name=name, opcode=row, uops=uops,
                rd1_en=_has_src1(spec)).sha(ver)
        return op

    # out = clip(in0 / in1, -imm2, imm2): one-Newton fast reciprocal of the
    # broadcast group-std (Src1) fused with the scale and the hardtanh clip.
    # 8/8 ALU stages; reciprocal rel err ~1.7e-3.
    from concourse.dve_spec import Bin, AluOp
    y0 = Bin(AluOp.BITWISE_NOT, Src1, Src1) * C0
    y1 = y0 * (C1 - Src1 * y0)
    # the clip reuses the Newton constant C1=2.0017324 as the bound (8-stage
    # budget): clipping at +-2.0017 instead of +-2.0 adds <=1.7e-3 abs error

    def _ref_apply(in0, in1, s0, s1, imm2):
        x = np.ascontiguousarray(in1.astype(np.float32))
        nx = (~x.view(np.int32)).view(np.float32)
        y0r = nx * s0
        y1r = y0r * (s1 - x * y0r)
        return np.minimum(np.maximum(in0.astype(np.float32) * y1r, -s1), s1)

    _custom_ops["apply"] = register("APPLY_RECIP_CLIP_ANT", Spec(
        body=minn(maxx(Src0 * y1, Zero - C1), C1),
        reference=_ref_apply))
    return _custom_ops


def build(m_loc: int, apply_affine: bool):
    import concourse.bass as bass
    import concourse.mybir as mybir
    import concourse.tile as tile
    from concourse import bacc
    from contextlib import ExitStack

    ops = _register_custom_ops()
    f32 = mybir.dt.float32
    f16 = mybir.dt.float16
    Alu = mybir.AluOpType
    n_tiles = m_loc // 128
    chunk = min(CHUNK, m_loc)
    tpc = chunk // 128  # m-tiles per x.T chunk
    grp = min(GRP, tpc)  # m-tiles per stats group
    n_chunks = m_loc // chunk

    nc = bacc.Bacc()
    xt_d = nc.dram_tensor("xt", [K, m_loc], f16, kind="ExternalInput")
    # packed weights: [128, 4096] = kt-major h0 halves then h1 halves
    wtp_d = nc.dram_tensor("wtp", [128, 2 * KT * 512], f16,
                           kind="ExternalInput")
    wgb_d = nc.dram_tensor("wgb", [128, KT * SW], f16, kind="ExternalInput")
    gb_d = nc.dram_tensor("gb", [SW, N], f16, kind="ExternalInput")
    msk_d = nc.dram_tensor("msk", [128, 1], f32, kind="ExternalInput")
    if apply_affine:
        gam_d = nc.dram_tensor("gam", [128, N], f32, kind="ExternalInput")
        bet_d = nc.dram_tensor("bet", [128, N], f32, kind="ExternalInput")
    out_d = nc.dram_tensor("out", [m_loc, N], f16, kind="ExternalOutput")

    with tile.TileContext(nc) as tc, ExitStack() as ctx:
        const = ctx.enter_context(tc.tile_pool(name="const", bufs=1))
        xpool = ctx.enter_context(tc.tile_pool(name="xts", bufs=3))
        pph0 = ctx.enter_context(tc.tile_pool(name="pph0", bufs=3,
                                              space="PSUM"))
        pph1 = ctx.enter_context(tc.tile_pool(name="pph1", bufs=3,
                                              space="PSUM"))
        pps = ctx.enter_context(tc.tile_pool(name="pps", bufs=2, space="PSUM"))
        epi = ctx.enter_context(tc.tile_pool(name="epi", bufs=3))
        extp = ctx.enter_context(tc.tile_pool(name="extp", bufs=2))
        outp = ctx.enter_context(tc.tile_pool(name="outp", bufs=4))

        # --- PE p-state warmup: zero matmuls keep the Tensor engine busy
        # (ramping its clock) while the first weight/x DMAs land ---
        warm_sb = const.tile([128, 512], f16, tag="warm")
        nc.vector.memset(warm_sb[:], 0.0)
        warm_ps = pps.tile([SW, 512], f32, tag="pt")
        for _ in range(N_WARM):
            nc.tensor.matmul(warm_ps[0:1, :], warm_sb[:, 0:1], warm_sb[:],
                             start=True, stop=True)

        # --- resident constants.  Packed DMAs keep the per-DMA ~620ns queue
        # dispatch cost off the startup critical path: the first main matmul
        # needs only wtp[:, 0:512] (kt0/h0) and the first x piece. ---
        wt_sb = const.tile([128, 2 * KT * 512], f16, tag="wt")
        nc.sync.dma_start(out=wt_sb[:, 0:512], in_=wtp_d[:, 0:512])
        nc.sync.dma_start(out=wt_sb[:, 512:2048], in_=wtp_d[:, 512:2048])

        xts0 = xpool.tile([128, KT * chunk], f16, tag="xts")
        x3_0 = xt_d[0:K, 0:512].rearrange("(k p) c -> p k c", p=128)
        nc.scalar.dma_start(
            out=xts0[:].rearrange("p (k c) -> p k c", c=chunk)[:, :, 0:512],
            in_=x3_0)
        wgb_sb = const.tile([128, KT * SW], f16, tag="wgb")
        nc.scalar.dma_start(out=wgb_sb[:], in_=wgb_d[:])
        # per-partition bias mask for the staging copy: 1.0 at the ones rows
        ones_sb = const.tile([128, 1], f32, tag="onesmask")
        nc.scalar.dma_start(out=ones_sb[:], in_=msk_d[:])
        # h1 weight halves (first h1 matmul is ~10 matmuls in)
        nc.sync.dma_start(out=wt_sb[:, 2048:4096], in_=wtp_d[:, 2048:4096])
        if chunk > 512:
            x3_1 = xt_d[0:K, 512:chunk].rearrange("(k p) c -> p k c", p=128)
            nc.scalar.dma_start(
                out=xts0[:].rearrange("p (k c) -> p k c",
                                      c=chunk)[:, :, 512:chunk],
                in_=x3_1)
        gb_sb = const.tile([SW, N], f16, tag="gb")
        nc.scalar.dma_start(out=gb_sb[:], in_=gb_d[:])
        eps_sb = const.tile([128, 1], f32, tag="eps")
        nc.vector.memset(eps_sb[:], EPS)
        if apply_affine:
            gam_sb = const.tile([128, N], f32, tag="gam")
            nc.scalar.dma_start(out=gam_sb[:], in_=gam_d[:])
            bet_sb = const.tile([128, N], f32, tag="bet")
            nc.scalar.dma_start(out=bet_sb[:], in_=bet_d[:])

        state_a = {}
        state_b = {}
        cur = {"xts": xts0, "next": None, "ext": None}

        def wslice(kt, h):
            return wt_sb[:, (h * KT + kt) * 512:(h * KT + kt + 1) * 512]

        def emit_main(mt):
            sc, loc = divmod(mt, tpc)
            if loc == 0 and sc > 0:
                cur["xts"] = cur["next"]
                cur["next"] = None
            if loc == max(0, tpc - 4) and sc + 1 < n_chunks:
                # prefetch next chunk as one packed DMA, four tiles early
                # (3-deep pool: the target buffer was freed a chunk ago)
                t = xpool.tile([128, KT * chunk], f16, tag="xts")
                x3 = xt_d[0:K, (sc + 1) * chunk:(sc + 2) * chunk].rearrange(
                    "(k p) c -> p k c", p=128)
                nc.sync.dma_start(
                    out=t[:].rearrange("p (k c) -> p k c", c=chunk), in_=x3)
                cur["next"] = t
            xts = cur["xts"]
            gloc = mt % grp
            lhsTs = [xts[:, kt * chunk + loc * 128:kt * chunk + (loc + 1) * 128]
                     for kt in range(KT)]
            ph0 = pph0.tile([128, 512], f32, tag="py0")
            ph1 = pph1.tile([128, 512], f32, tag="py1")
            for kt in range(KT):
                nc.tensor.matmul(ph0[:], lhsTs[kt], wslice(kt, 0),
                                 start=(kt == 0), stop=False)
            if gloc == 0:
                # group stats: one [49, 512] matmul set per 4 m-tiles
                pt = pps.tile([SW, 512], f32, tag="pt")
                g0 = loc * 128
                for kt in range(KT):
                    nc.tensor.matmul(
                        pt[:], wgb_sb[:, kt * SW:(kt + 1) * SW],
                        xts[:, kt * chunk + g0:kt * chunk + g0 + grp * 128],
                        start=(kt == 0), stop=(kt == KT - 1))
                # stage [-mean | 1] rows to SBUF fp16 for the injection:
                # Identity activation adds the constant ones rows via the
                # per-partition bias mask (stats rows of the mask are 0).
                ext = extp.tile([SW, grp * 128], f16, tag="ext")
                nc.scalar.activation(
                    out=ext[:], in_=pt[:],
                    func=mybir.ActivationFunctionType.Identity,
                    bias=ones_sb[0:SW, :], scale=1.0)
                cur["ext"] = ext
            for kt in range(KT):
                nc.tensor.matmul(ph1[:], lhsTs[kt], wslice(kt, 1),
                                 start=(kt == 0), stop=False)
            state_a[mt] = (ph0, ph1, cur["ext"], mt % grp)

        def emit_epi_a(mt):
            ph0, ph1, ext, gloc = state_a.pop(mt)
            exs = ext[:, gloc * 128:(gloc + 1) * 128]
            # inject (b - mean) into the y PSUM: rank-17 matmul per half
            nc.tensor.matmul(ph0[:], exs[0:17, :], gb_sb[0:17, 0:512],
                             start=False, stop=True)
            nc.tensor.matmul(ph1[:], exs[32:SW, :], gb_sb[32:SW, 512:N],
                             start=False, stop=True)
            # variance: square (Scalar, fp16 out) -> first pair-fold on
            # GpSimd -> second fold + segmented reduce on Vector (fp16)
            ysq = epi.tile([128, N], f16, tag="ysq")
            nc.scalar.square(ysq[:, 0:512], ph0[:])
            nc.scalar.square(ysq[:, 512:N], ph1[:])
            ysq3 = ysq[:].rearrange("p (g e) -> p g e", e=GS)
            t2 = epi.tile([128, N // 2], f16, tag="t2")
            nc.gpsimd.tensor_add(
                t2[:].rearrange("p (g e) -> p g e", e=GS // 2),
                ysq3[:, :, 0:GS // 2], ysq3[:, :, GS // 2:GS])
            state_b[mt] = (ph0, ph1, t2)

        def emit_epi_b(mt):
            ph0, ph1, t2 = state_b.pop(mt)
            t23 = t2[:].rearrange("p (g e) -> p g e", e=GS // 2)
            t4 = epi.tile([128, N // 4], f16, tag="t4")
            nc.vector.tensor_add(
                t4[:].rearrange("p (g e) -> p g e", e=GS // 4),
                t23[:, :, 0:GS // 4], t23[:, :, GS // 4:GS // 2])
            Q = epi.tile([128, NG], f16, tag="Q")
            with nc.allow_low_precision(reason="fp16 group sums of squares; "
                                        "var rel err ~1e-3 vs 2e-2 budget"):
                nc.vector.tensor_reduce(
                    out=Q[:],
                    in_=t4[:].rearrange("p (g e) -> p g e", e=GS // 4),
                    axis=mybir.AxisListType.X, op=Alu.add)
            # group std = sqrt(Q/32 + eps): scale+bias fold into the ACT sqrt
            s = epi.tile([128, NG], f32, tag="s")
            nc.scalar.activation(
                out=s[:], in_=Q[:], func=mybir.ActivationFunctionType.Sqrt,
                bias=eps_sb[:], scale=1.0 / GS)
            # apply per half: out = clip(y'/std, -2, 2), fused recip+clip;
            # h0's psum frees one apply earlier than h1's
            o = outp.tile([128, N], f16, tag="o")
            for h, ph in ((0, ph0), (1, ph1)):
                sh = bass.AP(tensor=s.tensor, offset=s.offset + 16 * h,
                             ap=[s.ap[0], [1, 16], [0, GS]])
                nc.vector._custom_dve(
                    ops["apply"],
                    out=o[:, 512 * h:512 * (h + 1)].rearrange(
                        "p (g e) -> p g e", e=GS),
                    in0=ph[:].rearrange("p (g e) -> p g e", e=GS),
                    in1=sh, s0=-0.23549792, s1=2.0017324)
            if apply_affine:
                nc.vector.tensor_mul(o[:], o[:], gam_sb[:])
                nc.vector.tensor_add(o[:], o[:], bet_sb[:])
                nc.vector.tensor_scalar(
                    out=o[:], in0=o[:], scalar1=-HT, scalar2=HT,
                    op0=Alu.max, op1=Alu.min)
            nc.sync.dma_start(out=out_d[mt * 128:(mt + 1) * 128, :], in_=o[:])

        # oldest-tile work first on every engine so short late-stage ops are
        # not queued behind long earlier-stage ops of newer tiles
        for mt in range(n_tiles):
            if mt >= 2:
                emit_epi_b(mt - 2)
            if mt >= 1:
                emit_epi_a(mt - 1)
            emit_main(mt)
        if n_tiles >= 2:
            emit_epi_b(n_tiles - 2)
        emit_epi_a(n_tiles - 1)
        emit_epi_b(n_tiles - 1)

    nc.finalize()
    return nc


def _prep_host(x, weight, bias, m_loc):
    bf = np.float16
    wtT = np.ascontiguousarray(weight.T.astype(bf))  # [K, N]
    # packed weights [128, 4096]: kt-major h0 halves then h1 halves
    wtp_h = np.zeros((128, 2 * KT * 512), dtype=bf)
    for kt in range(KT):
        wtp_h[:, kt * 512:(kt + 1) * 512] = wtT[kt * 128:(kt + 1) * 128,
                                                0:512]
        wtp_h[:, 2048 + kt * 512:2048 + (kt + 1) * 512] = \
            wtT[kt * 128:(kt + 1) * 128, 512:N]
    # stats stationary: per k-tile columns = -(1/32) * group-sum of weights,
    # already transposed ([K, group]); ones/pad columns stay 0.  Packed
    # kt-major into [128, KT*SW].
    wg = weight.reshape(NG, GS, K).sum(axis=1) * (-1.0 / GS)  # [NG, K]
    wgb_h = np.zeros((128, KT * SW), dtype=bf)
    for kt in range(KT):
        blk = np.zeros((128, SW), dtype=bf)
        blk[:, 0:16] = wg[0:16, kt * 128:(kt + 1) * 128].T.astype(bf)
        blk[:, 32:48] = wg[16:32, kt * 128:(kt + 1) * 128].T.astype(bf)
        wgb_h[:, kt * SW:(kt + 1) * SW] = blk
    # injection moving operand: group indicator rows + b'' rows
    b1 = bias.reshape(NG, GS).mean(axis=1)
    bpp = (bias - np.repeat(b1, GS)).astype(np.float64)
    gb_h = np.zeros((SW, N), dtype=bf)
    for g in range(16):
        gb_h[g, g * GS:(g + 1) * GS] = np.float16(1.0)
        gb_h[32 + g, 512 + g * GS:512 + (g + 1) * GS] = np.float16(1.0)
    gb_h[16, 0:512] = bpp[0:512].astype(bf)
    gb_h[48, 512:1024] = bpp[512:1024].astype(bf)
    msk_h = np.zeros((128, 1), dtype=np.float32)
    msk_h[16, 0] = 1.0
    msk_h[48, 0] = 1.0
    return wtp_h, wgb_h, gb_h, msk_h


def run(x, weight, bias, gamma, beta, m_loc=None, trace=False):
    from concourse.bass_utils import run_bass_kernel_spmd

    bf = np.float16
    x = np.asarray(x, dtype=np.float32)
    weight = np.asarray(weight, dtype=np.float32)
    bias = np.asarray(bias, dtype=np.float32)
    gamma = np.asarray(gamma, dtype=np.float32)
    beta = np.asarray(beta, dtype=np.float32)

    m_total = x.shape[0]
    if m_loc is None:
        m_loc = m_total // N_CORES
    assert m_total == m_loc * N_CORES

    apply_affine = not (np.all(gamma == 1.0) and np.all(beta == 0.0))
    nc = build(m_loc, apply_affine)
    wtp_h, wgb_h, gb_h, msk_h = _prep_host(x, weight, bias, m_loc)

    in_maps = []
    for c in range(N_CORES):
        m = {
            "xt": np.ascontiguousarray(
                x[c * m_loc:(c + 1) * m_loc, :].T.astype(bf)),
            "wtp": wtp_h, "wgb": wgb_h, "gb": gb_h, "msk": msk_h,
        }
        if apply_affine:
            m["gam"] = np.ascontiguousarray(np.broadcast_to(gamma, (128, N)))
            m["bet"] = np.ascontiguousarray(np.broadcast_to(beta, (128, N)))
        in_maps.append(m)

    res = run_bass_kernel_spmd(nc, in_maps, list(range(N_CORES)), trace=trace)
    out = np.concatenate([res.results[c]["out"] for c in range(N_CORES)],
                         axis=0).astype(np.float32)
    return out, res


def kernel(x, weight, bias, gamma, beta):
    out, _ = run(x, weight, bias, gamma, beta)
    return out
